# revision 23
# baseline (speedup 1.0000x reference)
"""Trainium2 Bass kernel for nn_CHAN_without_SA (conv/attention/deconv scorer).

Full-input contract: kernel(**inputs) takes the complete unsharded inputs,
shards data-parallel over batch*max_seg_num across 8 NeuronCores (10 sequences
per core; each core's sequences all belong to one batch element), runs one SPMD
Bass/Tile program, and reassembles the full output.

FP8 (e4m3) DoubleRow edition: conv1/conv2/deconv1/deconv2 matmuls run with
perf_mode=DoubleRow (2 fp8 weights per PE cell, contraction 256/instruction,
~1.4-2x bf16 throughput). Weights are pre-scaled by a power of two so their
~0.02-sigma values land in the fp8 normal range; activations are rescaled at
each stage output via the activation-unit scale operand (exact).

Layout tricks:
  - conv rhs streams a flat multi-sequence window (FD 408/412/508 >= 256) so
    the doubled LDWEIGHTS stays hidden behind the matmul; junk columns between
    sequence regions are simply never read out of PSUM.
  - deconv1's broadcast r-channels stay rank-1 (wsum/wcorr folded matmuls).
  - the SDIM=1024 final projection is folded to v = w_sim1^T((w_sim2 c) * w_mlp)
    exactly, per batch element.
"""
import numpy as np
import ml_dtypes

BF16 = ml_dtypes.bfloat16
F8 = ml_dtypes.float8_e4m3      # IEEE-style e4m3: matches TRN FP8_EXP4 on [0,240]

B, M, L = 4, 20, 200
IN_C, C1, C2 = 2048, 512, 256
CDIM, DC1, DC2, SDIM = 300, 512, 256, 1024
NEG = -1e15
Lq = L // 4           # 50
NCORES = 8
SEQ = 10              # sequences per core
PAIRS = 5
M1, M2 = 4, 2         # output 128-tiles for conv1 / conv2
MD1, MD2 = 4, 2       # output 128-tiles for deconv1 / deconv2
BLOCKS = [(0, 1), (2, 3), (4,)]   # pair blocks (conv2 batches 2 pairs)
# power-of-two weight scales: w*S lands sigma~0.3 in the fp8 normal range
S1, S2, SA, SD1, SD2 = 8.0, 16.0, 16.0, 16.0, 16.0


def _build_program():
    import concourse.bass as bass
    import concourse.mybir as mybir
    import concourse.tile as tile
    from concourse import bacc
    from contextlib import ExitStack

    dt = mybir.dt
    f32, bf16, f8 = dt.float32, dt.bfloat16, dt.float8e4
    AF = mybir.ActivationFunctionType
    ALU = mybir.AluOpType
    DR = mybir.MatmulPerfMode.DoubleRow
    X_AX = mybir.AxisListType.X

    nc = bacc.Bacc()
    P = nc.declare_dram_parameter
    d_xw = P("xw", [PAIRS, 128, 6656], f8, isOutput=False)
    d_w1 = P("w1", [8, 128, 5120], f8, isOutput=False)
    d_w2 = P("w2", [2, 128, 2560], f8, isOutput=False)
    d_wd1 = P("wd1", [128, 4096], f8, isOutput=False)
    d_wd2 = P("wd2", [2, 128, 2048], f8, isOutput=False)
    d_wsum = P("wsum", [128, 4096], f8, isOutput=False)
    d_wcorr = P("wcorr", [128, 4096], f8, isOutput=False)
    d_wca2 = P("wca2", [2, 128, 256], f8, isOutput=False)
    d_wca3 = P("wca3", [2, 128, 1], bf16, isOutput=False)
    d_qv = P("qv", [4, 128, 1], f32, isOutput=False)
    d_b1s = P("b1s", [M1, 128, 1], f32, isOutput=False)
    d_b2 = P("b2", [M2, 128, 1], f32, isOutput=False)
    d_bd1 = P("bd1", [MD1, 128, 1], f32, isOutput=False)
    d_bd2 = P("bd2", [MD2, 128, 1], f32, isOutput=False)
    d_mask = P("amask", [PAIRS, 1, 2 * Lq], f32, isOutput=False)
    d_v = P("vmat", [2, 128, 2], bf16, isOutput=False)
    d_bmlp = P("bmlp", [2, 1], f32, isOutput=False)
    d_bd1s = P("bd1s", [1, 512], f32, isOutput=False)   # SD1 * b_dc1, m-major
    d_out = P("out", [2, SEQ * L], f32, isOutput=True)

    with ExitStack() as ctx:
        tc = ctx.enter_context(tile.TileContext(nc))
        wp = ctx.enter_context(tc.tile_pool(name="weights", bufs=1))
        ap_ = ctx.enter_context(tc.tile_pool(name="acts", bufs=1))
        tp = ctx.enter_context(tc.tile_pool(name="trans", bufs=2))
        pp = ctx.enter_context(tc.tile_pool(name="psum", bufs=1, space="PSUM"))

        def apn(t, off, *dims):
            # free-dim view at element offset `off`: dims = (stride, n) pairs
            base = t[:, off:off + 1]
            return bass.AP(tensor=base.tensor, offset=base.offset,
                           ap=[base.ap[0]] + [list(d) for d in dims])

        # ---- bulk DMA on the two HWDGE issue engines, all issued up front
        # (tiles are persistent single-buffer: no reuse waits can ever park
        # in front of compute instructions on these queues)
        def wtile(eng, src, i, shape, dtyp, tag):
            t = wp.tile(shape, dtyp, tag=tag, name=tag)
            eng.dma_start(out=t, in_=src[i] if i is not None else src[:])
            return t

        def stile(eng, src, sl, shape, dtyp, tag):
            # bulk load of a free-dim slice of a dram param row
            t = wp.tile(shape, dtyp, tag=tag, name=tag)
            eng.dma_start(out=t, in_=src[:, sl])
            return t

        # x pairs 0/1 load in kp-pair chunks and w1 in per-m chunks (m-major
        # free layout [m(4), k01(2), t(5), co(128)]), ordered so DMA arrival
        # tracks conv1's consumption order. Only the head-critical subset is
        # issued up front; later groups are staged between conv1 m-passes so
        # their issue slots never park in front of pool ACTIVATEs.
        _rr = [0]

        def bulk_eng():
            _rr[0] += 1
            return nc.sync if _rr[0] % 2 == 0 else nc.scalar

        xk = {}          # (pair, chunk of 2 kp) -> [128, 1664] tile, pairs 0/1
        w1m = {}         # (kp, m) -> [128, 1280] tile
        xt = [None] * PAIRS

        def load_w1m(kp, m):
            w1m[kp, m] = stile(bulk_eng(), d_w1[kp],
                               slice(m * 1280, (m + 1) * 1280),
                               [128, 1280], f8, f"w1_{kp}m{m}")

        # pair-0's x and the m=0 weights first: conv1 runs pair-major, so the
        # very first accumulation chain touches only these
        for c in range(4):
            xk[0, c] = stile(bulk_eng(), d_xw[0],
                             slice(c * 1664, (c + 1) * 1664),
                             [128, 1664], f8, f"x0c{c}")
            load_w1m(2 * c, 0)
            load_w1m(2 * c + 1, 0)
        for c in range(4):
            xk[1, c] = stile(bulk_eng(), d_xw[1],
                             slice(c * 1664, (c + 1) * 1664),
                             [128, 1664], f8, f"x1c{c}")
            load_w1m(2 * c, 1)
            load_w1m(2 * c + 1, 1)
        wca2sb = [None, None]
        w2sb = [None, None]

        def dma_stage2():
            for kp in range(8):
                load_w1m(kp, 2)
            xt[2] = wtile(bulk_eng(), d_xw, 2, [128, 6656], f8, "x2")

        def dma_stage3():
            for kp in range(8):
                load_w1m(kp, 3)
            xt[3] = wtile(bulk_eng(), d_xw, 3, [128, 6656], f8, "x3")

        def dma_stage4():
            for k in range(2):
                wca2sb[k] = wtile(bulk_eng(), d_wca2, k, [128, 256], f8, f"wca2_{k}")
                w2sb[k] = wtile(bulk_eng(), d_w2, k, [128, 2560], f8, f"w2_{k}")

        # deconv-phase weights: needed only ~150us in. Tiles are declared now
        # but their loads are emitted in dma_stage3/4 so the transfers never
        # compete with the head-critical x/w1 stream.
        wd1sb = wp.tile([128, 4096], f8, tag="wd1", name="wd1")
        wd2sb = [wp.tile([128, 2048], f8, tag=f"wd2_{k}", name=f"wd2_{k}")
                 for k in range(2)]
        wsumsb = wp.tile([128, 4096], f8, tag="wsum", name="wsum")
        wcorrsb = wp.tile([128, 4096], f8, tag="wcorr", name="wcorr")

        def dma_stage5():
            xt[4] = wtile(bulk_eng(), d_xw, 4, [128, 6656], f8, "x4")
            bulk_eng().dma_start(out=wd1sb, in_=d_wd1[:])
            for k in range(2):
                bulk_eng().dma_start(out=wd2sb[k], in_=d_wd2[k])
            bulk_eng().dma_start(out=wsumsb, in_=d_wsum[:])
            bulk_eng().dma_start(out=wcorrsb, in_=d_wcorr[:])

        def w1ap(kp, m, t):
            return apn(w1m[kp, m], t * 128, (640, 2), (1, 128))

        def xap(p, kp, t):
            if p < 2:
                return apn(xk[p, kp // 2], (kp % 2) * 832 + t, (416, 2), (1, 408))
            return apn(xt[p], kp * 832 + t, (416, 2), (1, 408))

        # ---- persistent activations; pads are zeroed once, never rewritten
        t1p = [ap_.tile([128, 832], f8, tag=f"t1p{k}", name=f"t1p{k}")
               for k in range(2)]
        cat8 = ap_.tile([128, 1056], f8, tag="cat8", name="cat8")
        d1p8 = [ap_.tile([128, 2048], f8, tag=f"d1p{k}", name=f"d1p{k}")
                for k in range(2)]
        d2sb = [ap_.tile([128, SEQ * 200], bf16, tag=f"d2_{m}", name=f"d2_{m}")
                for m in range(MD2)]
        rcol = [[ap_.tile([128, SEQ], f32, tag=f"rcol{c}{k}", name=f"rcol{c}{k}")
                 for k in range(2)] for c in range(2)]
        for t in t1p + [cat8] + d1p8:
            nc.gpsimd.memset(t, 0.0)

        # small constants via SWDGE (one queue sem per DMA)
        b1s = [wtile(nc.gpsimd, d_b1s, m, [128, 1], f32, f"b1s_{m}") for m in range(M1)]
        b2 = [wtile(nc.gpsimd, d_b2, m, [128, 1], f32, f"b2_{m}") for m in range(M2)]
        qv = [wtile(nc.gpsimd, d_qv, i, [128, 1], f32, f"qv_{i}") for i in range(4)]
        wca3 = [wtile(nc.gpsimd, d_wca3, k, [128, 1], bf16, f"wca3_{k}") for k in range(2)]
        mkp = [wtile(nc.gpsimd, d_mask, p, [1, 2 * Lq], f32, f"mask{p}") for p in range(PAIRS)]
        bd1 = [wtile(nc.gpsimd, d_bd1, m, [128, 1], f32, f"bd1_{m}") for m in range(MD1)]
        bd2 = [wtile(nc.gpsimd, d_bd2, m, [128, 1], f32, f"bd2_{m}") for m in range(MD2)]
        vm = [wtile(nc.gpsimd, d_v, k, [128, 2], bf16, f"v_{k}") for k in range(2)]
        bmlp = wtile(nc.gpsimd, d_bmlp, None, [2, 1], f32, "bmlp")
        bd1s = wtile(nc.gpsimd, d_bd1s, None, [1, 512], f32, "bd1s")
        ones8 = ap_.tile([1, 8], f32, tag="ones8", name="ones8")
        nc.gpsimd.memset(ones8, 1.0)

        # ============ conv1: x[2048,200] -> maxpool -> t1[512,100] ==========
        # DoubleRow over channel-ktile pairs; rhs streams a flat 2-seq window
        # (FD=408, junk cols [200,208) per seq region). t1 is kept S1-scaled
        # in fp8 (pool max commutes with the affine S1*x + S1*b map).
        def emit_conv1(blk, weave=()):
            prs = BLOCKS[blk]
            for m in range(M1):
                for pi in range(len(prs)):
                    ps = pp.tile([128, 508], f32, tag="big", name="big", bufs=4)
                    for kp in range(8):
                        for t in range(5):
                            nc.tensor.matmul(
                                ps[:, 0:408], lhsT=w1ap(kp, m, t),
                                rhs=xap(prs[pi], kp, t),
                                start=(kp == 0 and t == 0), stop=(kp == 7 and t == 4),
                                perf_mode=DR)
                    pre = apn(ps, 0, (208, 2), (2, 100))
                    pro = apn(ps, 1, (208, 2), (2, 100))
                    tv = tp.tile([128, 200], f32, tag="ptmp1", name="ptmp1", bufs=3)
                    tvv = apn(tv, 0, (100, 2), (1, 100))
                    nc.scalar.activation(out=tvv, in_=pre, func=AF.Identity,
                                         bias=b1s[m], scale=1.0)
                    dst = apn(t1p[m // 2], (m % 2) * 416 + (2 * pi) * 104 + 2,
                              (104, 2), (1, 100))
                    nc.vector.scalar_tensor_tensor(
                        out=dst, in0=pro, scalar=b1s[m], in1=tvv,
                        op0=ALU.add, op1=ALU.max)
                if m < len(weave):
                    weave[m]()

        # ============ conv2 + maxpool -> t2 (cat8, true scale) ==============
        def emit_conv2(blk):
            prs = BLOCKS[blk]
            sblk = 2 * len(prs)
            W2 = 104 * (sblk - 1) + 100
            for m in range(M2):
                ps = pp.tile([128, 508], f32, tag="big", name="big", bufs=4)
                n = 0
                for kp in range(2):
                    for t in range(5):
                        nc.tensor.matmul(
                            ps[:, 0:W2],
                            lhsT=apn(w2sb[kp], t * 256 + m * 128, (1280, 2), (1, 128)),
                            rhs=apn(t1p[kp], t, (416, 2), (1, W2)),
                            start=(n == 0), stop=(n == 9), perf_mode=DR)
                        n += 1
                pre = apn(ps, 0, (104, sblk), (2, 50))
                pro = apn(ps, 1, (104, sblk), (2, 50))
                te = tp.tile([128, 200], f32, tag="c2e", name="c2e", bufs=2)
                to = tp.tile([128, 200], f32, tag="c2o", name="c2o", bufs=2)
                tev = apn(te, 0, (50, sblk), (1, 50))
                tov = apn(to, 0, (50, sblk), (1, 50))
                inv = 1.0 / (S1 * S2)
                nc.scalar.activation(out=tev, in_=pre, func=AF.Identity,
                                     bias=b2[m], scale=inv)
                nc.scalar.activation(out=tov, in_=pro, func=AF.Identity,
                                     bias=b2[m], scale=inv)
                dst = apn(cat8, m * 528 + (4 * blk) * 52 + 1, (52, sblk), (1, 50))
                nc.vector.tensor_max(dst, tev, tov)

        # ============ additive attention for one pair =======================
        def attention(p):
            # both kproj m-groups pack into one PSUM bank (disjoint columns)
            kpm = pp.tile([128, 200], f32, tag="mm200", name="mm200", bufs=1)
            kp_ps = []
            for m in range(M2):
                sl = kpm[:, m * 100:(m + 1) * 100]
                for k in range(2):
                    nc.tensor.matmul(
                        sl, lhsT=wca2sb[k][:, m * 128:(m + 1) * 128],
                        rhs=apn(cat8, k * 528 + 104 * p + 1, (52, 2), (1, 50)),
                        start=(k == 0), stop=(k == 1))
                kp_ps.append(sl)
            for c in range(2):
                th = []
                for m in range(M2):
                    thm = tp.tile([128, 100], bf16, tag=f"th{c}{m}", name=f"th{c}{m}", bufs=2)
                    nc.scalar.activation(out=thm, in_=kp_ps[m], func=AF.Tanh,
                                         bias=qv[2 * c + m], scale=1.0 / SA)
                    th.append(thm)
                spt = pp.tile([2, 400], f32, tag="fin", name="fin", bufs=2)
                sp = spt[0:1, 0:100]
                for m in range(M2):
                    nc.tensor.matmul(sp, lhsT=wca3[m], rhs=th[m],
                                     start=(m == 0), stop=(m == 1))
                # masked softmax, unnormalized exp: |scores| <= ~2 so no
                # max-subtraction needed; masked lanes are exp(-1e15) = 0
                def bc2(t):
                    return bass.AP(tensor=t.tensor, offset=t.offset,
                                   ap=[t.ap[0], [1, 2], [0, Lq]])
                sm = tp.tile([1, 100], f32, tag="sm", name="sm", bufs=4)
                nc.vector.tensor_add(sm, sp[0:1, 0:100], mkp[p])
                ex = tp.tile([1, 100], f32, tag="ex", name="ex", bufs=4)
                nc.scalar.activation(out=ex, in_=sm, func=AF.Exp,
                                     bias=0.0, scale=1.0)
                exv = ex.rearrange("q (s l) -> q s l", s=2)
                se = tp.tile([1, 2], f32, tag="se", name="se", bufs=4)
                nc.vector.tensor_reduce(out=se, in_=exv, axis=X_AX, op=ALU.add)
                rc = tp.tile([1, 2], f32, tag="rc", name="rc", bufs=4)
                nc.vector.reciprocal(rc, se)
                av = tp.tile([1, 100], f32, tag="av", name="av", bufs=4)
                nc.vector.tensor_mul(av.rearrange("q (s l) -> q s l", s=2),
                                     exv, bc2(rc))
                abc2 = tp.tile([128, 100], f32, tag="abc", name="abc", bufs=4)
                nc.gpsimd.partition_broadcast(abc2, av[0:1, :], channels=128)
                for s01 in range(2):
                    s = 2 * p + s01
                    abc = abc2[:, Lq * s01:Lq * s01 + Lq]
                    for k in range(2):
                        scr = tp.tile([128, Lq], f32, tag="rscr", name="rscr", bufs=2)
                        nc.vector.tensor_mul(
                            scr, apn(cat8, k * 528 + 52 * s + 1, (1, Lq)), abc)
                        nc.vector.tensor_reduce(
                            out=rcol[c][k][:, s:s + 1], in_=scr,
                            axis=X_AX, op=ALU.add)

        # ============ tail: deconv1/deconv2/final, in two 5-seq stages ======
        # Stage g=0 (seqs 0-4, ready after pair-2 attention) runs its matmuls
        # while pair-4's attention DVE chain drains — the PE never idles long
        # enough to re-throttle HAM mid-tail.
        rcolb = [ap_.tile([128, SEQ], f8, tag=f"rcolb{c}{k}", name=f"rcolb{c}{k}")
                 for c in range(2) for k in range(2)]   # rk = 2*c + k
        rt = pp.tile([128, 160], f32, tag="rt", name="rt", bufs=1)
        TAPS = [[(1, 1), (3, 0)], [(2, 1), (0, 2)]]   # (tap, rhs offset) per phase

        def emit_rt(gi):
            # r-channels of deconv1 are rank-1 per sequence: fold to matmuls.
            # A rank-1 f32 matmul adds SD1*b_dc1 into the r half so the ert
            # unpack below is one batched activation with bias=0.
            for c in range(2):
                for k in range(2):
                    nc.vector.tensor_copy(out=rcolb[2 * c + k][:, 5 * gi:5 * gi + 5],
                                          in_=rcol[c][k][:, 5 * gi:5 * gi + 5])
            for ph in range(2):
                for m in range(MD1):
                    base = gi * 80 + (ph * 4 + m) * 10
                    for half, wsb in ((0, wsumsb), (5, wcorrsb)):
                        for rk in range(4):
                            o = (ph * 4 + rk) * 512 + m * 128
                            nc.tensor.matmul(
                                rt[:, base + half:base + half + 5],
                                lhsT=wsb[:, o:o + 128],
                                rhs=rcolb[rk][:, 5 * gi:5 * gi + 5],
                                start=(rk == 0), stop=(rk == 3 and half == 5))
                        if half == 0:
                            nc.tensor.matmul(
                                rt[:, base:base + 5],
                                lhsT=bd1s[0:1, m * 128:(m + 1) * 128],
                                rhs=ones8[0:1, 0:5], start=False, stop=True)

        def emit_deconv1(gi):
            # cat[512ch,50] -> d1[512,100] over the per-position t2 channels;
            # ert r-terms for all 8 (ph,m) groups unpack in single batched ops
            ertm = tp.tile([128, 40], f32, tag="ertm", name="ertm", bufs=2)
            nc.scalar.activation(out=apn(ertm, 0, (5, 8), (1, 5)),
                                 in_=apn(rt, gi * 80, (10, 8), (1, 5)),
                                 func=AF.Identity, bias=0.0, scale=1.0 / SD1)
            ertc = tp.tile([128, 40], f32, tag="ertc", name="ertc", bufs=2)
            nc.scalar.activation(out=apn(ertc, 0, (5, 8), (1, 5)),
                                 in_=apn(rt, gi * 80 + 5, (10, 8), (1, 5)),
                                 func=AF.Identity, bias=0.0, scale=1.0 / SD1)
            er2 = tp.tile([128, 40], f32, tag="er2", name="er2", bufs=2)
            nc.vector.tensor_sub(er2, ertm, ertc)
            for m in range(MD1):
                for ph in range(2):
                    g8 = (ph * 4 + m) * 5
                    psd = pp.tile([128, 508], f32, tag="big", name="big", bufs=4)
                    for ti, (t, off) in enumerate(TAPS[ph]):
                        nc.tensor.matmul(
                            psd[:, 0:258],
                            lhsT=apn(wd1sb, t * 512 + m * 128, (2048, 2), (1, 128)),
                            rhs=apn(cat8, 260 * gi + off, (528, 2), (1, 258)),
                            start=(ti == 0), stop=(ti == 1), perf_mode=DR)
                    base = (m % 2) * 1024 + 510 * gi + 1 + ph
                    nc.vector.scalar_tensor_tensor(
                        out=apn(d1p8[m // 2], base, (102, 5), (2, 50)),
                        in0=apn(psd, 0, (52, 5), (1, 50)),
                        scalar=1.0 / SD1,
                        in1=apn(ertm, g8, (1, 5), (0, 50)),
                        op0=ALU.mult, op1=ALU.add)
                    bcol = 0 if ph == 0 else Lq - 1
                    nc.vector.scalar_tensor_tensor(
                        out=apn(d1p8[m // 2], base + 2 * bcol, (102, 5), (2, 1)),
                        in0=apn(psd, bcol, (52, 5), (1, 1)),
                        scalar=1.0 / SD1,
                        in1=apn(er2, g8, (1, 5), (0, 1)),
                        op0=ALU.mult, op1=ALU.add)

        def emit_deconv2(gi):
            # d1[512,100] -> d2[256,200]; kp-outer accumulation so the kp=0
            # half starts as soon as deconv1's m0/m1 outputs land
            for m in range(MD2):
                for ph in range(2):
                    psd = pp.tile([128, 508], f32, tag="big", name="big", bufs=4)
                    n = 0
                    for kp in range(2):
                        for t, off in TAPS[ph]:
                            nc.tensor.matmul(
                                psd,
                                lhsT=apn(wd2sb[kp], t * 256 + m * 128, (1024, 2), (1, 128)),
                                rhs=apn(d1p8[kp], 510 * gi + off, (1024, 2), (1, 508)),
                                start=(n == 0), stop=(n == 3), perf_mode=DR)
                            n += 1
                    nc.scalar.activation(
                        out=apn(d2sb[m], (5 * gi) * 200 + ph, (200, 5), (2, 100)),
                        in_=apn(psd, 0, (102, 5), (1, 100)),
                        func=AF.Identity, bias=bd2[m], scale=1.0 / SD2)

        def emit_final(p):
            # folded projection v.d2 + sigmoid
            fp = pp.tile([2, 400], f32, tag="fin", name="fin", bufs=2)
            for k in range(2):
                nc.tensor.matmul(fp, lhsT=vm[k],
                                 rhs=d2sb[k][:, 400 * p:400 * (p + 1)],
                                 start=(k == 0), stop=(k == 1))
            fo = tp.tile([2, 400], f32, tag="fout", name="fout", bufs=5)
            nc.scalar.activation(out=fo, in_=fp, func=AF.Sigmoid,
                                 bias=bmlp, scale=1.0)
            nc.sync.dma_start(out=d_out[:, 400 * p:400 * (p + 1)], in_=fo)

        # ---- pipeline: previous block's attention weaves into the next
        # block's conv1 m-passes so score matmuls never stall the PE;
        # blk0's weave slots stage the later bulk DMA issues instead
        emit_conv1(0, weave=(dma_stage2, dma_stage3, dma_stage4, dma_stage5))
        emit_conv2(0)
        emit_conv1(1, weave=(lambda: attention(0), lambda: attention(1)))
        emit_conv2(1)
        # group-0 tail work (seqs 0-4, ready once pair-2's attention lands)
        # weaves into blk2's conv1 so the PE stays dense through the tail
        emit_conv1(2, weave=(lambda: attention(2), lambda: attention(3),
                             lambda: emit_rt(0), lambda: emit_deconv1(0)))
        emit_conv2(2)
        attention(4)
        emit_deconv2(0)
        emit_final(0)
        emit_final(1)
        emit_rt(1)
        emit_deconv1(1)
        emit_deconv2(1)
        emit_final(2)
        emit_final(3)
        emit_final(4)

    nc.compile()   # bacc legalization: splits sync waits to <=1 per inst
    return nc


def _prep_inputs(batch, seg_len, concept1, concept2,
                 w_conv1, b_conv1, w_conv2, b_conv2,
                 w_ca1, w_ca2, w_ca3,
                 w_dc1, b_dc1, w_dc2, b_dc2,
                 w_sim1, w_sim2, w_mlp, b_mlp):
    f32 = np.float32

    # x: [B,M,L,IN_C] -> per core [PAIRS, 128, (kp8, k01, s01, 208)] fp8
    bm = np.ascontiguousarray(batch, f32).reshape(B * M, L, IN_C)
    bt = bm.transpose(0, 2, 1)                          # [80, 2048, 200]
    X = np.zeros((B * M, 16, 128, 208), F8)
    X[:, :, :, 2:202] = bt.reshape(B * M, 16, 128, L).astype(F8)
    xw = X.reshape(NCORES, PAIRS, 2, 8, 2, 128, 208) \
          .transpose(0, 1, 5, 3, 4, 2, 6).reshape(NCORES, PAIRS, 128, 6656)
    xw = np.ascontiguousarray(xw)

    # DoubleRow weight layouts, scaled; w1 is m-major [m, k01, t, co] so the
    # head-critical kp0/kp1 tiles can load in per-m chunks
    w1p = np.ascontiguousarray(
        (np.asarray(w_conv1, f32) * S1).reshape(M1, 128, 8, 2, 128, 5)
        .transpose(2, 4, 0, 3, 5, 1).reshape(8, 128, 5120)).astype(F8)
    w2p = np.ascontiguousarray(
        (np.asarray(w_conv2, f32) * S2).reshape(M2, 128, 2, 2, 128, 5)
        .transpose(2, 4, 3, 5, 0, 1).reshape(2, 128, 2560)).astype(F8)
    wd1_ = np.asarray(w_dc1, f32)
    wd1p = np.ascontiguousarray(
        (wd1_[:256] * SD1).reshape(2, 128, MD1, 128, 4)
        .transpose(1, 0, 4, 2, 3).reshape(128, 4096)).astype(F8)
    wd2p = np.ascontiguousarray(
        (np.asarray(w_dc2, f32) * SD2).reshape(2, 2, 128, MD2, 128, 4)
        .transpose(0, 2, 1, 5, 3, 4).reshape(2, 128, 2048)).astype(F8)
    # summed-tap / correction-tap deconv1 weights for the rank-1 r-channels
    wr = wd1_[256:768].reshape(4, 128, MD1, 128, 4)     # [rk, ci, m, co, t]
    wsum = np.ascontiguousarray(
        (np.stack([wr[..., 1] + wr[..., 3], wr[..., 2] + wr[..., 0]], 0) * SD1)
        .transpose(2, 0, 1, 3, 4).reshape(128, 4096)).astype(F8)
    wcorr = np.ascontiguousarray(
        (np.stack([wr[..., 3], wr[..., 0]], 0) * SD1)
        .transpose(2, 0, 1, 3, 4).reshape(128, 4096)).astype(F8)
    wca2p = np.ascontiguousarray(
        (np.asarray(w_ca2, f32).T * SA).reshape(2, 128, 256)).astype(F8)
    wca3t = np.asarray(w_ca3, f32)[0].reshape(2, 128, 1).astype(BF16)
    b1s = (S1 * np.asarray(b_conv1, f32)).reshape(M1, 128, 1)
    b2v = np.asarray(b_conv2, f32).reshape(M2, 128, 1)
    bd1v = np.asarray(b_dc1, f32).reshape(MD1, 128, 1)
    bd2v = np.asarray(b_dc2, f32).reshape(MD2, 128, 1)
    bmlp = np.full((2, 1), np.asarray(b_mlp, f32).reshape(-1)[0], f32)

    # per-core mask / q / v
    nvalid = ((np.asarray(seg_len) + 3) // 4).reshape(B * M)
    amask = np.where(np.arange(Lq)[None, :] < nvalid[:, None], 0.0, NEG) \
        .astype(f32).reshape(NCORES, PAIRS, 1, 2 * Lq)
    concepts = [np.asarray(concept1, f32), np.asarray(concept2, f32)]
    w_ca1 = np.asarray(w_ca1, f32)
    w_sim1 = np.asarray(w_sim1, f32)
    w_sim2 = np.asarray(w_sim2, f32)
    wm = np.asarray(w_mlp, f32)[0]
    qv_all = np.zeros((NCORES, 4, 128, 1), f32)
    v_all = np.zeros((NCORES, 2, 128, 2), f32)
    for core in range(NCORES):
        bidx = (core * SEQ) // M
        for c in range(2):
            q = w_ca1 @ concepts[c][bidx]                       # [256]
            qv_all[core, 2 * c:2 * c + 2] = q.reshape(2, 128, 1)
            v = w_sim1.T @ ((w_sim2 @ concepts[c][bidx]) * wm)  # [256]
            v_all[core, :, :, c] = v.reshape(2, 128)
    vmat = v_all.astype(BF16)

    shared = dict(w1=w1p, w2=w2p, wd1=wd1p, wd2=wd2p, wsum=wsum, wcorr=wcorr,
                  wca2=wca2p, wca3=wca3t, b1s=b1s, b2=b2v, bd1=bd1v, bd2=bd2v,
                  bmlp=bmlp, bd1s=(SD1 * np.asarray(b_dc1, f32)).reshape(1, 512))
    return [dict(shared, xw=xw[c], amask=amask[c], qv=qv_all[c], vmat=vmat[c])
            for c in range(NCORES)]


_CACHE = {}


def kernel(**inputs):
    from concourse.bass_utils import run_bass_kernel_spmd

    in_maps = _prep_inputs(**inputs)
    if "nc" not in _CACHE:
        _CACHE["nc"] = _build_program()
    res = run_bass_kernel_spmd(_CACHE["nc"], in_maps, list(range(NCORES)))
    out = np.stack([np.asarray(r["out"], np.float32) for r in res.results])
    sc = out.transpose(1, 0, 2).reshape(2, B, M, L)
    return sc[0], sc[1]


# revision 26
# speedup vs baseline: 1.0429x; 1.0429x over previous
"""Trainium2 Bass kernel for nn_CHAN_without_SA (conv/attention/deconv scorer).

Full-input contract: kernel(**inputs) takes the complete unsharded inputs,
shards data-parallel over batch*max_seg_num across 8 NeuronCores (10 sequences
per core; each core's sequences all belong to one batch element), runs one SPMD
Bass/Tile program, and reassembles the full output.

FP8 (e4m3) DoubleRow edition: conv1/conv2/deconv1/deconv2 matmuls run with
perf_mode=DoubleRow (2 fp8 weights per PE cell, contraction 256/instruction,
~1.4-2x bf16 throughput). Weights are pre-scaled by a power of two so their
~0.02-sigma values land in the fp8 normal range; activations are rescaled at
each stage output via the activation-unit scale operand (exact).

Layout tricks:
  - conv rhs streams a flat multi-sequence window (FD 408/412/508 >= 256) so
    the doubled LDWEIGHTS stays hidden behind the matmul; junk columns between
    sequence regions are simply never read out of PSUM.
  - deconv1's broadcast r-channels stay rank-1 (wsum/wcorr folded matmuls).
  - the SDIM=1024 final projection is folded to v = w_sim1^T((w_sim2 c) * w_mlp)
    exactly, per batch element.
"""
import numpy as np
import ml_dtypes

BF16 = ml_dtypes.bfloat16
F8 = ml_dtypes.float8_e4m3      # IEEE-style e4m3: matches TRN FP8_EXP4 on [0,240]

B, M, L = 4, 20, 200
IN_C, C1, C2 = 2048, 512, 256
CDIM, DC1, DC2, SDIM = 300, 512, 256, 1024
NEG = -1e15
Lq = L // 4           # 50
NCORES = 8
SEQ = 10              # sequences per core
PAIRS = 5
M1, M2 = 4, 2         # output 128-tiles for conv1 / conv2
MD1, MD2 = 4, 2       # output 128-tiles for deconv1 / deconv2
BLOCKS = [(0, 1), (2, 3), (4,)]   # pair blocks (conv2 batches 2 pairs)
# power-of-two weight scales: w*S lands sigma~0.3 in the fp8 normal range
S1, S2, SA, SD1, SD2 = 8.0, 16.0, 16.0, 16.0, 16.0


def _build_program():
    import concourse.bass as bass
    import concourse.mybir as mybir
    import concourse.tile as tile
    from concourse import bacc
    from contextlib import ExitStack

    dt = mybir.dt
    f32, bf16, f8 = dt.float32, dt.bfloat16, dt.float8e4
    AF = mybir.ActivationFunctionType
    ALU = mybir.AluOpType
    DR = mybir.MatmulPerfMode.DoubleRow
    X_AX = mybir.AxisListType.X

    nc = bacc.Bacc()
    P = nc.declare_dram_parameter
    d_xw = P("xw", [PAIRS, 128, 6656], f8, isOutput=False)
    d_w1 = P("w1", [8, 128, 5120], f8, isOutput=False)
    d_w2 = P("w2", [2, 128, 2560], f8, isOutput=False)
    d_wd1 = P("wd1", [128, 4096], f8, isOutput=False)
    d_wd2 = P("wd2", [2, 128, 2048], f8, isOutput=False)
    d_wsum = P("wsum", [128, 4096], f8, isOutput=False)
    d_wcorr = P("wcorr", [128, 4096], f8, isOutput=False)
    d_wca2 = P("wca2", [2, 128, 256], f8, isOutput=False)
    d_wca3 = P("wca3", [2, 128, 1], bf16, isOutput=False)
    d_qv = P("qv", [4, 128, 1], f32, isOutput=False)
    d_b1s = P("b1s", [M1, 128, 1], f32, isOutput=False)
    d_b2 = P("b2", [M2, 128, 1], f32, isOutput=False)
    d_bd1 = P("bd1", [MD1, 128, 1], f32, isOutput=False)
    d_bd2 = P("bd2", [MD2, 128, 1], f32, isOutput=False)
    d_mask = P("amask", [PAIRS, 1, 2 * Lq], f32, isOutput=False)
    d_v = P("vmat", [2, 128, 2], bf16, isOutput=False)
    d_bmlp = P("bmlp", [2, 1], f32, isOutput=False)
    d_bd1s = P("bd1s", [1, 512], f32, isOutput=False)   # SD1 * b_dc1, m-major
    d_out = P("out", [2, SEQ * L], f32, isOutput=True)

    with ExitStack() as ctx:
        tc = ctx.enter_context(tile.TileContext(nc))
        wp = ctx.enter_context(tc.tile_pool(name="weights", bufs=1))
        ap_ = ctx.enter_context(tc.tile_pool(name="acts", bufs=1))
        tp = ctx.enter_context(tc.tile_pool(name="trans", bufs=2))
        pp = ctx.enter_context(tc.tile_pool(name="psum", bufs=1, space="PSUM"))

        def apn(t, off, *dims):
            # free-dim view at element offset `off`: dims = (stride, n) pairs
            base = t[:, off:off + 1]
            return bass.AP(tensor=base.tensor, offset=base.offset,
                           ap=[base.ap[0]] + [list(d) for d in dims])

        # ---- bulk DMA on the two HWDGE issue engines, all issued up front
        # (tiles are persistent single-buffer: no reuse waits can ever park
        # in front of compute instructions on these queues)
        def wtile(eng, src, i, shape, dtyp, tag):
            t = wp.tile(shape, dtyp, tag=tag, name=tag)
            eng.dma_start(out=t, in_=src[i] if i is not None else src[:])
            return t

        def stile(eng, src, sl, shape, dtyp, tag):
            # bulk load of a free-dim slice of a dram param row
            t = wp.tile(shape, dtyp, tag=tag, name=tag)
            eng.dma_start(out=t, in_=src[:, sl])
            return t

        # x pairs 0/1 load in kp-pair chunks and w1 in per-m chunks (m-major
        # free layout [m(4), k01(2), t(5), co(128)]), ordered so DMA arrival
        # tracks conv1's consumption order. Only the head-critical subset is
        # issued up front; later groups are staged between conv1 m-passes so
        # their issue slots never park in front of pool ACTIVATEs.
        _rr = [0]

        def bulk_eng():
            _rr[0] += 1
            return nc.sync if _rr[0] % 2 == 0 else nc.scalar

        xk = {}          # (pair, chunk of 2 kp) -> [128, 1664] tile, pairs 0/1
        w1m = {}         # (kp, m) -> [128, 1280] tile
        xt = [None] * PAIRS

        def load_w1m(kp, m):
            w1m[kp, m] = stile(bulk_eng(), d_w1[kp],
                               slice(m * 1280, (m + 1) * 1280),
                               [128, 1280], f8, f"w1_{kp}m{m}")

        # pair-0's x and the m=0 weights first: conv1 runs pair-major, so the
        # very first accumulation chain touches only these
        for c in range(4):
            xk[0, c] = stile(bulk_eng(), d_xw[0],
                             slice(c * 1664, (c + 1) * 1664),
                             [128, 1664], f8, f"x0c{c}")
            load_w1m(2 * c, 0)
            load_w1m(2 * c + 1, 0)
        for c in range(4):
            xk[1, c] = stile(bulk_eng(), d_xw[1],
                             slice(c * 1664, (c + 1) * 1664),
                             [128, 1664], f8, f"x1c{c}")
            load_w1m(2 * c, 1)
            load_w1m(2 * c + 1, 1)
        wca2sb = [None, None]
        w2sb = [None, None]

        def dma_stage2():
            for kp in range(8):
                load_w1m(kp, 2)
            xt[2] = wtile(bulk_eng(), d_xw, 2, [128, 6656], f8, "x2")

        def dma_stage3():
            for kp in range(8):
                load_w1m(kp, 3)
            xt[3] = wtile(bulk_eng(), d_xw, 3, [128, 6656], f8, "x3")

        def dma_stage4():
            for k in range(2):
                wca2sb[k] = wtile(bulk_eng(), d_wca2, k, [128, 256], f8, f"wca2_{k}")
                w2sb[k] = wtile(bulk_eng(), d_w2, k, [128, 2560], f8, f"w2_{k}")

        # deconv-phase weights: needed only ~150us in. Tiles are declared now
        # but their loads are emitted in dma_stage3/4 so the transfers never
        # compete with the head-critical x/w1 stream.
        wd1sb = wp.tile([128, 4096], f8, tag="wd1", name="wd1")
        wd2sb = [wp.tile([128, 2048], f8, tag=f"wd2_{k}", name=f"wd2_{k}")
                 for k in range(2)]
        wsumsb = wp.tile([128, 4096], f8, tag="wsum", name="wsum")
        wcorrsb = wp.tile([128, 4096], f8, tag="wcorr", name="wcorr")

        def dma_stage5():
            xt[4] = wtile(bulk_eng(), d_xw, 4, [128, 6656], f8, "x4")
            bulk_eng().dma_start(out=wd1sb, in_=d_wd1[:])
            for k in range(2):
                bulk_eng().dma_start(out=wd2sb[k], in_=d_wd2[k])
            bulk_eng().dma_start(out=wsumsb, in_=d_wsum[:])
            bulk_eng().dma_start(out=wcorrsb, in_=d_wcorr[:])

        def w1ap(kp, m, t):
            return apn(w1m[kp, m], t * 128, (640, 2), (1, 128))

        def xap(p, kp, t):
            if p < 2:
                return apn(xk[p, kp // 2], (kp % 2) * 832 + t, (416, 2), (1, 408))
            return apn(xt[p], kp * 832 + t, (416, 2), (1, 408))

        # ---- persistent activations; pads are zeroed once, never rewritten
        t1p = [ap_.tile([128, 832], f8, tag=f"t1p{k}", name=f"t1p{k}")
               for k in range(2)]
        cat8 = ap_.tile([128, 1056], f8, tag="cat8", name="cat8")
        d1p8 = [ap_.tile([128, 2048], f8, tag=f"d1p{k}", name=f"d1p{k}")
                for k in range(2)]
        d2sb = [ap_.tile([128, SEQ * 200], bf16, tag=f"d2_{m}", name=f"d2_{m}")
                for m in range(MD2)]
        rcol = [[ap_.tile([128, SEQ], f32, tag=f"rcol{c}{k}", name=f"rcol{c}{k}")
                 for k in range(2)] for c in range(2)]
        for t in t1p + [cat8] + d1p8:
            nc.gpsimd.memset(t, 0.0)

        # small constants via SWDGE (one queue sem per DMA)
        b1s = [wtile(nc.gpsimd, d_b1s, m, [128, 1], f32, f"b1s_{m}") for m in range(M1)]
        b2 = [wtile(nc.gpsimd, d_b2, m, [128, 1], f32, f"b2_{m}") for m in range(M2)]
        qv = [wtile(nc.gpsimd, d_qv, i, [128, 1], f32, f"qv_{i}") for i in range(4)]
        wca3 = [wtile(nc.gpsimd, d_wca3, k, [128, 1], bf16, f"wca3_{k}") for k in range(2)]
        mkp = [wtile(nc.gpsimd, d_mask, p, [1, 2 * Lq], f32, f"mask{p}") for p in range(PAIRS)]
        bd1 = [wtile(nc.gpsimd, d_bd1, m, [128, 1], f32, f"bd1_{m}") for m in range(MD1)]
        bd2 = [wtile(nc.gpsimd, d_bd2, m, [128, 1], f32, f"bd2_{m}") for m in range(MD2)]
        vm = [wtile(nc.gpsimd, d_v, k, [128, 2], bf16, f"v_{k}") for k in range(2)]
        bmlp = wtile(nc.gpsimd, d_bmlp, None, [2, 1], f32, "bmlp")
        bd1s = wtile(nc.gpsimd, d_bd1s, None, [1, 512], f32, "bd1s")
        ones8 = ap_.tile([1, 8], f32, tag="ones8", name="ones8")
        nc.gpsimd.memset(ones8, 1.0)

        # ---- PE warm-up: junk matmuls on the zeroed t1p while the head DMAs
        # stream in, so HAM un-throttles (K=8/8) before the first real matmul
        warm = pp.tile([128, 200], f32, tag="mm200", name="mm200", bufs=1)
        for _ in range(8):
            nc.tensor.matmul(warm, lhsT=t1p[0][:, 0:128], rhs=t1p[0][:, 0:200],
                             start=True, stop=True)

        # ============ conv1: x[2048,200] -> maxpool -> t1[512,100] ==========
        # DoubleRow over channel-ktile pairs; rhs streams a flat 2-seq window
        # (FD=408, junk cols [200,208) per seq region). t1 is kept S1-scaled
        # in fp8 (pool max commutes with the affine S1*x + S1*b map).
        def emit_conv1(blk, weave=()):
            prs = BLOCKS[blk]
            for m in range(M1):
                for pi in range(len(prs)):
                    ps = pp.tile([128, 508], f32, tag="big", name="big", bufs=4)
                    for kp in range(8):
                        for t in range(5):
                            nc.tensor.matmul(
                                ps[:, 0:408], lhsT=w1ap(kp, m, t),
                                rhs=xap(prs[pi], kp, t),
                                start=(kp == 0 and t == 0), stop=(kp == 7 and t == 4),
                                perf_mode=DR)
                    pre = apn(ps, 0, (208, 2), (2, 100))
                    pro = apn(ps, 1, (208, 2), (2, 100))
                    tv = tp.tile([128, 200], f32, tag="ptmp1", name="ptmp1", bufs=3)
                    tvv = apn(tv, 0, (100, 2), (1, 100))
                    nc.scalar.activation(out=tvv, in_=pre, func=AF.Identity,
                                         bias=b1s[m], scale=1.0)
                    dst = apn(t1p[m // 2], (m % 2) * 416 + (2 * pi) * 104 + 2,
                              (104, 2), (1, 100))
                    nc.vector.scalar_tensor_tensor(
                        out=dst, in0=pro, scalar=b1s[m], in1=tvv,
                        op0=ALU.add, op1=ALU.max)
                if m < len(weave):
                    weave[m]()

        # ============ conv2 + maxpool -> t2 (cat8, true scale) ==============
        def emit_conv2(blk):
            prs = BLOCKS[blk]
            sblk = 2 * len(prs)
            W2 = 104 * (sblk - 1) + 100
            for m in range(M2):
                ps = pp.tile([128, 508], f32, tag="big", name="big", bufs=4)
                n = 0
                for kp in range(2):
                    for t in range(5):
                        nc.tensor.matmul(
                            ps[:, 0:W2],
                            lhsT=apn(w2sb[kp], t * 256 + m * 128, (1280, 2), (1, 128)),
                            rhs=apn(t1p[kp], t, (416, 2), (1, W2)),
                            start=(n == 0), stop=(n == 9), perf_mode=DR)
                        n += 1
                pre = apn(ps, 0, (104, sblk), (2, 50))
                pro = apn(ps, 1, (104, sblk), (2, 50))
                te = tp.tile([128, 200], f32, tag="c2e", name="c2e", bufs=2)
                to = tp.tile([128, 200], f32, tag="c2o", name="c2o", bufs=2)
                tev = apn(te, 0, (50, sblk), (1, 50))
                tov = apn(to, 0, (50, sblk), (1, 50))
                inv = 1.0 / (S1 * S2)
                nc.scalar.activation(out=tev, in_=pre, func=AF.Identity,
                                     bias=b2[m], scale=inv)
                nc.scalar.activation(out=tov, in_=pro, func=AF.Identity,
                                     bias=b2[m], scale=inv)
                dst = apn(cat8, m * 528 + (4 * blk) * 52 + 1, (52, sblk), (1, 50))
                nc.vector.tensor_max(dst, tev, tov)

        # ============ additive attention for one pair =======================
        def attention(p):
            # both kproj m-groups pack into one PSUM bank (disjoint columns)
            kpm = pp.tile([128, 200], f32, tag="mm200", name="mm200", bufs=1)
            kp_ps = []
            for m in range(M2):
                sl = kpm[:, m * 100:(m + 1) * 100]
                for k in range(2):
                    nc.tensor.matmul(
                        sl, lhsT=wca2sb[k][:, m * 128:(m + 1) * 128],
                        rhs=apn(cat8, k * 528 + 104 * p + 1, (52, 2), (1, 50)),
                        start=(k == 0), stop=(k == 1))
                kp_ps.append(sl)
            for c in range(2):
                th = []
                for m in range(M2):
                    thm = tp.tile([128, 100], bf16, tag=f"th{c}{m}", name=f"th{c}{m}", bufs=2)
                    nc.scalar.activation(out=thm, in_=kp_ps[m], func=AF.Tanh,
                                         bias=qv[2 * c + m], scale=1.0 / SA)
                    th.append(thm)
                spt = pp.tile([2, 400], f32, tag="fin", name="fin", bufs=2)
                sp = spt[0:1, 0:100]
                for m in range(M2):
                    nc.tensor.matmul(sp, lhsT=wca3[m], rhs=th[m],
                                     start=(m == 0), stop=(m == 1))
                # masked softmax, unnormalized exp: |scores| <= ~2 so no
                # max-subtraction needed; masked lanes are exp(-1e15) = 0
                def bc2(t):
                    return bass.AP(tensor=t.tensor, offset=t.offset,
                                   ap=[t.ap[0], [1, 2], [0, Lq]])
                sm = tp.tile([1, 100], f32, tag="sm", name="sm", bufs=4)
                nc.vector.tensor_add(sm, sp[0:1, 0:100], mkp[p])
                ex = tp.tile([1, 100], f32, tag="ex", name="ex", bufs=4)
                nc.scalar.activation(out=ex, in_=sm, func=AF.Exp,
                                     bias=0.0, scale=1.0)
                exv = ex.rearrange("q (s l) -> q s l", s=2)
                se = tp.tile([1, 2], f32, tag="se", name="se", bufs=4)
                nc.vector.tensor_reduce(out=se, in_=exv, axis=X_AX, op=ALU.add)
                rc = tp.tile([1, 2], f32, tag="rc", name="rc", bufs=4)
                nc.vector.reciprocal(rc, se)
                av = tp.tile([1, 100], f32, tag="av", name="av", bufs=4)
                nc.vector.tensor_mul(av.rearrange("q (s l) -> q s l", s=2),
                                     exv, bc2(rc))
                abc2 = tp.tile([128, 100], f32, tag="abc", name="abc", bufs=4)
                nc.gpsimd.partition_broadcast(abc2, av[0:1, :], channels=128)
                for s01 in range(2):
                    s = 2 * p + s01
                    abc = abc2[:, Lq * s01:Lq * s01 + Lq]
                    for k in range(2):
                        scr = tp.tile([128, Lq], f32, tag="rscr", name="rscr", bufs=2)
                        nc.vector.tensor_mul(
                            scr, apn(cat8, k * 528 + 52 * s + 1, (1, Lq)), abc)
                        nc.vector.tensor_reduce(
                            out=rcol[c][k][:, s:s + 1], in_=scr,
                            axis=X_AX, op=ALU.add)

        # ============ tail: deconv1/deconv2/final, in two 5-seq stages ======
        # Stage g=0 (seqs 0-4, ready after pair-2 attention) runs its matmuls
        # while pair-4's attention DVE chain drains — the PE never idles long
        # enough to re-throttle HAM mid-tail.
        rcolb = [ap_.tile([128, SEQ], f8, tag=f"rcolb{c}{k}", name=f"rcolb{c}{k}")
                 for c in range(2) for k in range(2)]   # rk = 2*c + k
        rt = pp.tile([128, 160], f32, tag="rt", name="rt", bufs=1)
        TAPS = [[(1, 1), (3, 0)], [(2, 1), (0, 2)]]   # (tap, rhs offset) per phase

        def emit_rt(gi):
            # r-channels of deconv1 are rank-1 per sequence: fold to matmuls.
            # A rank-1 f32 matmul adds SD1*b_dc1 into the r half so the ert
            # unpack below is one batched activation with bias=0.
            for c in range(2):
                for k in range(2):
                    nc.vector.tensor_copy(out=rcolb[2 * c + k][:, 5 * gi:5 * gi + 5],
                                          in_=rcol[c][k][:, 5 * gi:5 * gi + 5])
            for ph in range(2):
                for m in range(MD1):
                    base = gi * 80 + (ph * 4 + m) * 10
                    for half, wsb in ((0, wsumsb), (5, wcorrsb)):
                        for rk in range(4):
                            o = (ph * 4 + rk) * 512 + m * 128
                            nc.tensor.matmul(
                                rt[:, base + half:base + half + 5],
                                lhsT=wsb[:, o:o + 128],
                                rhs=rcolb[rk][:, 5 * gi:5 * gi + 5],
                                start=(rk == 0), stop=(rk == 3 and half == 5))
                        if half == 0:
                            nc.tensor.matmul(
                                rt[:, base:base + 5],
                                lhsT=bd1s[0:1, m * 128:(m + 1) * 128],
                                rhs=ones8[0:1, 0:5], start=False, stop=True)

        def emit_deconv1(gi):
            # cat[512ch,50] -> d1[512,100] over the per-position t2 channels;
            # ert r-terms for all 8 (ph,m) groups unpack in single batched ops
            ertm = tp.tile([128, 40], f32, tag="ertm", name="ertm", bufs=2)
            nc.scalar.activation(out=apn(ertm, 0, (5, 8), (1, 5)),
                                 in_=apn(rt, gi * 80, (10, 8), (1, 5)),
                                 func=AF.Identity, bias=0.0, scale=1.0 / SD1)
            ertc = tp.tile([128, 40], f32, tag="ertc", name="ertc", bufs=2)
            nc.scalar.activation(out=apn(ertc, 0, (5, 8), (1, 5)),
                                 in_=apn(rt, gi * 80 + 5, (10, 8), (1, 5)),
                                 func=AF.Identity, bias=0.0, scale=1.0 / SD1)
            er2 = tp.tile([128, 40], f32, tag="er2", name="er2", bufs=2)
            nc.vector.tensor_sub(er2, ertm, ertc)
            for m in range(MD1):
                for ph in range(2):
                    g8 = (ph * 4 + m) * 5
                    psd = pp.tile([128, 508], f32, tag="big", name="big", bufs=4)
                    for ti, (t, off) in enumerate(TAPS[ph]):
                        nc.tensor.matmul(
                            psd[:, 0:258],
                            lhsT=apn(wd1sb, t * 512 + m * 128, (2048, 2), (1, 128)),
                            rhs=apn(cat8, 260 * gi + off, (528, 2), (1, 258)),
                            start=(ti == 0), stop=(ti == 1), perf_mode=DR)
                    base = (m % 2) * 1024 + 510 * gi + 1 + ph
                    nc.vector.scalar_tensor_tensor(
                        out=apn(d1p8[m // 2], base, (102, 5), (2, 50)),
                        in0=apn(psd, 0, (52, 5), (1, 50)),
                        scalar=1.0 / SD1,
                        in1=apn(ertm, g8, (1, 5), (0, 50)),
                        op0=ALU.mult, op1=ALU.add)
                    bcol = 0 if ph == 0 else Lq - 1
                    nc.vector.scalar_tensor_tensor(
                        out=apn(d1p8[m // 2], base + 2 * bcol, (102, 5), (2, 1)),
                        in0=apn(psd, bcol, (52, 5), (1, 1)),
                        scalar=1.0 / SD1,
                        in1=apn(er2, g8, (1, 5), (0, 1)),
                        op0=ALU.mult, op1=ALU.add)

        def emit_deconv2(gi):
            # d1[512,100] -> d2[256,200]; kp-outer accumulation so the kp=0
            # half starts as soon as deconv1's m0/m1 outputs land
            for m in range(MD2):
                for ph in range(2):
                    psd = pp.tile([128, 508], f32, tag="big", name="big", bufs=4)
                    n = 0
                    for kp in range(2):
                        for t, off in TAPS[ph]:
                            nc.tensor.matmul(
                                psd,
                                lhsT=apn(wd2sb[kp], t * 256 + m * 128, (1024, 2), (1, 128)),
                                rhs=apn(d1p8[kp], 510 * gi + off, (1024, 2), (1, 508)),
                                start=(n == 0), stop=(n == 3), perf_mode=DR)
                            n += 1
                    nc.scalar.activation(
                        out=apn(d2sb[m], (5 * gi) * 200 + ph, (200, 5), (2, 100)),
                        in_=apn(psd, 0, (102, 5), (1, 100)),
                        func=AF.Identity, bias=bd2[m], scale=1.0 / SD2)

        foall = ap_.tile([2, SEQ * L], f32, tag="foall", name="foall")

        def emit_final(p):
            # folded projection v.d2 + sigmoid; one batched output DMA at the
            # end keeps the epilogue's queue-drain handshakes to a single queue
            fp = pp.tile([2, 400], f32, tag="fin", name="fin", bufs=2)
            for k in range(2):
                nc.tensor.matmul(fp, lhsT=vm[k],
                                 rhs=d2sb[k][:, 400 * p:400 * (p + 1)],
                                 start=(k == 0), stop=(k == 1))
            nc.scalar.activation(out=foall[:, 400 * p:400 * (p + 1)], in_=fp,
                                 func=AF.Sigmoid, bias=bmlp, scale=1.0)
            if p == PAIRS - 1:
                nc.sync.dma_start(out=d_out[:], in_=foall)

        # ---- pipeline: previous block's attention weaves into the next
        # block's conv1 m-passes so score matmuls never stall the PE;
        # blk0's weave slots stage the later bulk DMA issues instead
        emit_conv1(0, weave=(dma_stage2, dma_stage3, dma_stage4, dma_stage5))
        emit_conv2(0)
        emit_conv1(1, weave=(lambda: attention(0), lambda: attention(1)))
        emit_conv2(1)
        emit_conv1(2, weave=(lambda: attention(2), lambda: attention(3)))
        emit_conv2(2)
        # group-0 tail work (seqs 0-4) is the PE's cover while pair-4's
        # attention chain drains on ACT/DVE
        emit_rt(0)
        attention(4)
        emit_deconv1(0)
        emit_deconv2(0)
        emit_final(0)
        emit_final(1)
        emit_rt(1)
        emit_deconv1(1)
        emit_deconv2(1)
        emit_final(2)
        emit_final(3)
        emit_final(4)

    nc.compile()   # bacc legalization: splits sync waits to <=1 per inst
    return nc


def _prep_inputs(batch, seg_len, concept1, concept2,
                 w_conv1, b_conv1, w_conv2, b_conv2,
                 w_ca1, w_ca2, w_ca3,
                 w_dc1, b_dc1, w_dc2, b_dc2,
                 w_sim1, w_sim2, w_mlp, b_mlp):
    f32 = np.float32

    # x: [B,M,L,IN_C] -> per core [PAIRS, 128, (kp8, k01, s01, 208)] fp8
    bm = np.ascontiguousarray(batch, f32).reshape(B * M, L, IN_C)
    bt = bm.transpose(0, 2, 1)                          # [80, 2048, 200]
    X = np.zeros((B * M, 16, 128, 208), F8)
    X[:, :, :, 2:202] = bt.reshape(B * M, 16, 128, L).astype(F8)
    xw = X.reshape(NCORES, PAIRS, 2, 8, 2, 128, 208) \
          .transpose(0, 1, 5, 3, 4, 2, 6).reshape(NCORES, PAIRS, 128, 6656)
    xw = np.ascontiguousarray(xw)

    # DoubleRow weight layouts, scaled; w1 is m-major [m, k01, t, co] so the
    # head-critical kp0/kp1 tiles can load in per-m chunks
    w1p = np.ascontiguousarray(
        (np.asarray(w_conv1, f32) * S1).reshape(M1, 128, 8, 2, 128, 5)
        .transpose(2, 4, 0, 3, 5, 1).reshape(8, 128, 5120)).astype(F8)
    w2p = np.ascontiguousarray(
        (np.asarray(w_conv2, f32) * S2).reshape(M2, 128, 2, 2, 128, 5)
        .transpose(2, 4, 3, 5, 0, 1).reshape(2, 128, 2560)).astype(F8)
    wd1_ = np.asarray(w_dc1, f32)
    wd1p = np.ascontiguousarray(
        (wd1_[:256] * SD1).reshape(2, 128, MD1, 128, 4)
        .transpose(1, 0, 4, 2, 3).reshape(128, 4096)).astype(F8)
    wd2p = np.ascontiguousarray(
        (np.asarray(w_dc2, f32) * SD2).reshape(2, 2, 128, MD2, 128, 4)
        .transpose(0, 2, 1, 5, 3, 4).reshape(2, 128, 2048)).astype(F8)
    # summed-tap / correction-tap deconv1 weights for the rank-1 r-channels
    wr = wd1_[256:768].reshape(4, 128, MD1, 128, 4)     # [rk, ci, m, co, t]
    wsum = np.ascontiguousarray(
        (np.stack([wr[..., 1] + wr[..., 3], wr[..., 2] + wr[..., 0]], 0) * SD1)
        .transpose(2, 0, 1, 3, 4).reshape(128, 4096)).astype(F8)
    wcorr = np.ascontiguousarray(
        (np.stack([wr[..., 3], wr[..., 0]], 0) * SD1)
        .transpose(2, 0, 1, 3, 4).reshape(128, 4096)).astype(F8)
    wca2p = np.ascontiguousarray(
        (np.asarray(w_ca2, f32).T * SA).reshape(2, 128, 256)).astype(F8)
    wca3t = np.asarray(w_ca3, f32)[0].reshape(2, 128, 1).astype(BF16)
    b1s = (S1 * np.asarray(b_conv1, f32)).reshape(M1, 128, 1)
    b2v = np.asarray(b_conv2, f32).reshape(M2, 128, 1)
    bd1v = np.asarray(b_dc1, f32).reshape(MD1, 128, 1)
    bd2v = np.asarray(b_dc2, f32).reshape(MD2, 128, 1)
    bmlp = np.full((2, 1), np.asarray(b_mlp, f32).reshape(-1)[0], f32)

    # per-core mask / q / v
    nvalid = ((np.asarray(seg_len) + 3) // 4).reshape(B * M)
    amask = np.where(np.arange(Lq)[None, :] < nvalid[:, None], 0.0, NEG) \
        .astype(f32).reshape(NCORES, PAIRS, 1, 2 * Lq)
    concepts = [np.asarray(concept1, f32), np.asarray(concept2, f32)]
    w_ca1 = np.asarray(w_ca1, f32)
    w_sim1 = np.asarray(w_sim1, f32)
    w_sim2 = np.asarray(w_sim2, f32)
    wm = np.asarray(w_mlp, f32)[0]
    qv_all = np.zeros((NCORES, 4, 128, 1), f32)
    v_all = np.zeros((NCORES, 2, 128, 2), f32)
    for core in range(NCORES):
        bidx = (core * SEQ) // M
        for c in range(2):
            q = w_ca1 @ concepts[c][bidx]                       # [256]
            qv_all[core, 2 * c:2 * c + 2] = q.reshape(2, 128, 1)
            v = w_sim1.T @ ((w_sim2 @ concepts[c][bidx]) * wm)  # [256]
            v_all[core, :, :, c] = v.reshape(2, 128)
    vmat = v_all.astype(BF16)

    shared = dict(w1=w1p, w2=w2p, wd1=wd1p, wd2=wd2p, wsum=wsum, wcorr=wcorr,
                  wca2=wca2p, wca3=wca3t, b1s=b1s, b2=b2v, bd1=bd1v, bd2=bd2v,
                  bmlp=bmlp, bd1s=(SD1 * np.asarray(b_dc1, f32)).reshape(1, 512))
    return [dict(shared, xw=xw[c], amask=amask[c], qv=qv_all[c], vmat=vmat[c])
            for c in range(NCORES)]


_CACHE = {}


def kernel(**inputs):
    from concourse.bass_utils import run_bass_kernel_spmd

    in_maps = _prep_inputs(**inputs)
    if "nc" not in _CACHE:
        _CACHE["nc"] = _build_program()
    res = run_bass_kernel_spmd(_CACHE["nc"], in_maps, list(range(NCORES)))
    out = np.stack([np.asarray(r["out"], np.float32) for r in res.results])
    sc = out.transpose(1, 0, 2).reshape(2, B, M, L)
    return sc[0], sc[1]
